# revision 26
# baseline (speedup 1.0000x reference)
"""EvolveGCN-reg Trainium2 kernel (8 NeuronCores, timestep-parallel).

Math: the reference computes, per timestep t (scan carrying a GRU-evolved
16x16 weight W):
    y_t   = X_t @ p / ||p||;  (yk, idx) = top16(y_t);  Xs = (X_t[idx] * yk).T
    W_t   = GRU(W_{t-1}, Xs)          (16x16 matmuls, tiny)
    AH    = segment_sum(val * X_t[col], row, N)       (3.2M-edge sparse op)
    out_t = (AH @ W_t) @ lin_w + b
Key identity used here:  out_t = A_t @ (X_t @ (W_t @ lin_w)) + b, so the
feature dimension collapses and the sparse phase is a scalar gather /
segment-sum:  out_t[n] = b + sum_{e: row[e]=n} val[e] * s_t[col[e]],
with s_t = X_t @ u_t and u_t = W_t @ lin_w.

Sharding: core t owns timestep t (uniform load, no collectives). Host does
index-space layout only (the sharding hint's "partition edge lists by
destination range" taken to its conclusion): edges are grouped by
destination, destinations degree-sorted and assigned round-robin to the 128
SBUF partitions so each "rank" of 128 nodes shares a common padded segment
length. The segment-sum then becomes ~45 strided DVE tensor_reduce ops at
line rate. The one index permutation (s gathered edge-wise) happens during
host-side re-staging between device launches; every floating-point
operation of the model runs on the NeuronCores.

Device launches (all math on device):
  L1: yraw_t = X_t @ p                     -> host extracts top-16 *indices*
  L2: ||p|| normalize, Xs_tau, full GRU chain, u_t select (one-hot input
      mask), s_t = X_t @ u_t
  L3: w = val * s[col] (gathered layout), per-rank segmented reduce, + b
"""

import numpy as np
from contextlib import ExitStack

import concourse.bass as bass
import concourse.bacc as bacc
import concourse.tile as tile
from concourse import mybir
from concourse.bass_utils import run_bass_kernel_spmd

dt = mybir.dt

T, N, E, F0, F1 = 8, 100000, 3200000, 16, 16
NCORES = 8
P = 128
RANKS = (N + P - 1) // P  # 782
N_PAD = P * RANKS  # 100096
CORE_IDS = list(range(NCORES))

_cache = {}


def _axon_reset():
    try:
        import ctypes

        lib = ctypes.CDLL("/opt/axon/libaxon_pjrt.so")
        lib.axon_reset.restype = ctypes.c_int64
        lib.axon_reset()
    except Exception:
        pass


def _run(nc, in_maps):
    try:
        return run_bass_kernel_spmd(nc, in_maps, core_ids=CORE_IDS)
    except Exception:
        _axon_reset()
        return run_bass_kernel_spmd(nc, in_maps, core_ids=CORE_IDS)

GP_FEATS = (4, 5, 6, 7)  # features accumulated on GpSimd (rest on DVE)


def _emit_matvec(nc, io, acc_pool, xt_ap, u_sb, out_sb):
    """out_sb[p, r] = sum_f X_T[p, f, r] * u_sb[p, f]; X_T streamed in 8
    feature-chunks, fused multiply-accumulate chain on DVE."""
    xt = io.tile([P, F0, RANKS], dt.float32, tag="xt", name="xt")
    FC = F0 // 8
    for c4 in range(8):
        nc.sync.dma_start(
            xt[:, c4 * FC : (c4 + 1) * FC, :],
            xt_ap[:, c4 * FC * RANKS : (c4 + 1) * FC * RANKS].rearrange(
                "p (f r) -> p f r", r=RANKS),
        )
    # two interleaved accumulator chains hide DVE issue gaps
    acc_a = acc_pool.tile([P, RANKS], dt.float32, tag="acc_a", name="acc_a")
    acc_b = acc_pool.tile([P, RANKS], dt.float32, tag="acc_b", name="acc_b")
    nc.vector.tensor_scalar_mul(acc_a[:], xt[:, 0, :], u_sb[:, 0:1])
    nc.vector.tensor_scalar_mul(acc_b[:], xt[:, 1, :], u_sb[:, 1:2])
    for f in range(2, F0):
        acc = acc_a if f % 2 == 0 else acc_b
        nc.vector.scalar_tensor_tensor(
            out=acc[:], in0=xt[:, f, :], scalar=u_sb[:, f : f + 1],
            in1=acc[:], op0=mybir.AluOpType.mult, op1=mybir.AluOpType.add,
        )
    nc.vector.tensor_tensor(out=out_sb[:], in0=acc_a[:], in1=acc_b[:],
                            op=mybir.AluOpType.add)


# ---------------------------------------------------------------- launch 1
def _build_p1():
    nc = bacc.Bacc("TRN2", target_bir_lowering=False, debug=False)
    xt_ap = nc.dram_tensor("XT", [P, F0 * RANKS], dt.float32, kind="ExternalInput").ap()
    prep_ap = nc.dram_tensor("prep", [P, F0], dt.float32, kind="ExternalInput").ap()
    y_ap = nc.dram_tensor("yraw", [P, RANKS], dt.float32, kind="ExternalOutput").ap()

    with tile.TileContext(nc) as tc, ExitStack() as ctx:
        io = ctx.enter_context(tc.tile_pool(name="io", bufs=1))
        yp = ctx.enter_context(tc.tile_pool(name="y", bufs=1))
        p_t = yp.tile([P, F0], dt.float32)
        nc.scalar.dma_start(p_t[:], prep_ap[:])
        y_t = yp.tile([P, RANKS], dt.float32)
        _emit_matvec(nc, io, yp, xt_ap, p_t, y_t)
        nc.sync.dma_start(y_ap[:], y_t[:])
    nc.compile()
    return nc


# ---------------------------------------------------------------- launch 2
# packed small-input column layout (16 partition rows). Gate accumulators
# are PSUM-preloaded with the raw bias B (ACT copy), then two K=16 matmuls
# accumulate U@W and W@Xs on top -- no I16 bias matmul, no stacked rhs.
_COLS = {}
_off = 0
for _n, _w in [("UZT", 16), ("WZT", 16), ("BZR8", 256), ("URT", 16), ("WRT", 16),
               ("UHT", 16), ("WHT", 16), ("BH8", 128), ("X16", 128),
               ("YR16", 8), ("I16", 16), ("WINIT", 16), ("LINW", 16),
               ("SEL8", 8), ("PREP", 16)]:
    _COLS[_n] = (_off, _off + _w)
    _off += _w
SMALLS_W = _off


def _build_p2():
    nc = bacc.Bacc("TRN2", target_bir_lowering=False, debug=False)
    bf = dt.bfloat16
    xt_ap = nc.dram_tensor("XTB", [P, F0 * RANKS], bf, kind="ExternalInput").ap()
    sm_ap = nc.dram_tensor("smalls", [16, SMALLS_W], dt.float32, kind="ExternalInput").ap()
    s_ap = nc.dram_tensor("s", [P, RANKS], bf, kind="ExternalOutput").ap()

    with tile.TileContext(nc) as tc, ExitStack() as ctx:
        small = ctx.enter_context(tc.tile_pool(name="small", bufs=1))
        gru = ctx.enter_context(tc.tile_pool(name="gru", bufs=2))
        rhp = ctx.enter_context(tc.tile_pool(name="rhp", bufs=1))
        ps = ctx.enter_context(tc.tile_pool(name="ps", bufs=3, space="PSUM"))
        ps1 = ctx.enter_context(tc.tile_pool(name="ps1", bufs=1, space="PSUM"))
        io = ctx.enter_context(tc.tile_pool(name="io", bufs=1))
        sp = ctx.enter_context(tc.tile_pool(name="s", bufs=1))

        sm = small.tile([16, SMALLS_W], dt.float32)
        nc.scalar.dma_start(sm[:], sm_ap[:])

        # X (bf16) streamed in 4 feature chunks; queue the DMAs up front
        FCH = 4
        xtb = []
        for c in range(F0 // FCH):
            xt = io.tile([P, FCH, RANKS], bf, tag=f"xt{c}", name=f"xt{c}")
            nc.sync.dma_start(
                xt[:],
                xt_ap[:, c * FCH * RANKS : (c + 1) * FCH * RANKS].rearrange(
                    "p (f r) -> p f r", r=RANKS),
            )
            xtb.append(xt)

        def gi(name, rows=16):
            a, b = _COLS[name]
            return sm[0:rows, a:b]

        # invp = 1/||p||. square on DVE; sqrt FIRST so its ACT table load
        # lands before the sigmoid/tanh warms (table thrash costs 1.3us)
        psq = small.tile([1, F0], dt.float32)
        nc.vector.tensor_tensor(out=psq[:], in0=gi("PREP")[0:1, :],
                                in1=gi("PREP")[0:1, :], op=mybir.AluOpType.mult)
        pss = small.tile([1, 1], dt.float32)
        nc.vector.tensor_reduce(out=pss[:], in_=psq[:], axis=mybir.AxisListType.X,
                                op=mybir.AluOpType.add)
        pnorm = small.tile([1, 1], dt.float32)
        nc.scalar.sqrt(pnorm[:], pss[:])
        invp = small.tile([1, 1], dt.float32)
        nc.vector.reciprocal(invp[:], pnorm[:])
        # prefetch the chain's ACT tables; reading pnorm forces these AFTER
        # sqrt so its table load cannot evict sigmoid/tanh mid-kernel
        warm = small.tile([1, 2], dt.float32)
        nc.scalar.activation(warm[:, 0:1], pnorm[:],
                             mybir.ActivationFunctionType.Sigmoid)
        nc.scalar.activation(warm[:, 1:2], pnorm[:],
                             mybir.ActivationFunctionType.Tanh)
        ones1x16 = small.tile([1, 16], dt.float32)
        nc.vector.memset(ones1x16[:], 1.0)
        invp16_ps = ps1.tile([16, 1], dt.float32, tag="misc", name="invp16_ps")
        nc.tensor.matmul(invp16_ps[:], ones1x16[:], invp[:], start=True, stop=True)
        invp16 = small.tile([16, 1], dt.float32)
        nc.scalar.copy(invp16[:], invp16_ps[:])

        # W chain tiles + Xs tiles, all plain base-0 [16,16]
        Wt = [rhp.tile([16, 16], dt.float32, tag=f"w{t}", name=f"w{t}")
              for t in range(T + 1)]
        Xs = [rhp.tile([16, 16], dt.float32, tag=f"xs{t}", name=f"xs{t}")
              for t in range(T)]
        nc.scalar.copy(Wt[0][:], gi("WINIT"))

        # Xs_tau = X16_tau^T @ (I * yraw_tau * invp)
        for tau in range(T):
            dg = gru.tile([16, 16], dt.float32, tag="diag", name=f"dg{tau}")
            nc.vector.tensor_scalar(
                out=dg[:], in0=gi("I16"), scalar1=gi("YR16")[:, tau : tau + 1],
                op0=mybir.AluOpType.mult, scalar2=invp16[:],
                op1=mybir.AluOpType.mult)
            xs_ps = ps.tile([16, 16], dt.float32, tag="m16", name=f"xsps{tau}")
            nc.tensor.matmul(xs_ps[:],
                             gi("X16")[:, tau * F0 : (tau + 1) * F0],
                             dg[:], start=True, stop=True)
            nc.scalar.copy(Xs[tau][:], xs_ps[:])

        # GRU chain. All-tau PSUM accumulators: azr [16, 8*32] (Z|R per
        # tau), ah [16, 8*16]. Bias + W_gate@Xs parts are accumulated EARLY
        # (during the X DMA); per tau only U@W / U@RW + activation remain.
        SIG = mybir.ActivationFunctionType.Sigmoid
        TANH = mybir.ActivationFunctionType.Tanh
        u_cols = small.tile([16, T], dt.float32)
        azr = ps1.tile([16, T * 32], dt.float32, tag="azr", name="azr")
        ah = ps1.tile([16, T * 16], dt.float32, tag="ah", name="ah")
        nc.vector.tensor_copy(azr[:], gi("BZR8"))
        nc.vector.tensor_copy(ah[:], gi("BH8"))
        for tau in range(T):
            z0 = tau * 32
            nc.tensor.matmul(azr[:, z0 : z0 + 16], gi("WZT"), Xs[tau][:],
                             start=False, stop=False, skip_group_check=True)
            nc.tensor.matmul(azr[:, z0 + 16 : z0 + 32], gi("WRT"), Xs[tau][:],
                             start=False, stop=False, skip_group_check=True)
            nc.tensor.matmul(ah[:, tau * 16 : (tau + 1) * 16], gi("WHT"),
                             Xs[tau][:], start=False, stop=False,
                             skip_group_check=True)
        for tau in range(T):
            z0 = tau * 32
            nc.tensor.matmul(azr[:, z0 : z0 + 16], gi("UZT"), Wt[tau][:],
                             start=False, stop=False, skip_group_check=True)
            nc.tensor.matmul(azr[:, z0 + 16 : z0 + 32], gi("URT"), Wt[tau][:],
                             start=False, stop=True, skip_group_check=True)
            ZR = gru.tile([16, 32], dt.float32, tag="gzr", name=f"gzr{tau}")
            nc.scalar.activation(ZR[:], azr[:, z0 : z0 + 32], SIG)
            RW = gru.tile([16, 16], dt.float32, tag="rw", name=f"rw{tau}")
            nc.vector.tensor_tensor(out=RW[:], in0=ZR[:, 16:32], in1=Wt[tau][:],
                                    op=mybir.AluOpType.mult)
            nc.tensor.matmul(ah[:, tau * 16 : (tau + 1) * 16], gi("UHT"),
                             RW[:], start=False, stop=True,
                             skip_group_check=True)
            # W_new = (W - Zg*W) + Zg*Ht; the first half is ready right
            # after sigmoid, so only two DVE ops trail the tanh
            zw = gru.tile([16, 16], dt.float32, tag="zw", name=f"zw{tau}")
            nc.vector.tensor_tensor(out=zw[:], in0=ZR[:, 0:16], in1=Wt[tau][:],
                                    op=mybir.AluOpType.mult)
            wm = gru.tile([16, 16], dt.float32, tag="wm", name=f"wm{tau}")
            nc.vector.tensor_tensor(out=wm[:], in0=Wt[tau][:], in1=zw[:],
                                    op=mybir.AluOpType.subtract)
            Ht = gru.tile([16, 16], dt.float32, tag="ht", name=f"ht{tau}")
            nc.scalar.activation(Ht[:], ah[:, tau * 16 : (tau + 1) * 16], TANH)
            zh = gru.tile([16, 16], dt.float32, tag="zh", name=f"zh{tau}")
            nc.vector.tensor_tensor(out=zh[:], in0=ZR[:, 0:16], in1=Ht[:],
                                    op=mybir.AluOpType.mult)
            nc.vector.tensor_tensor(out=Wt[tau + 1][:], in0=wm[:],
                                    in1=zh[:], op=mybir.AluOpType.add)
            # u_tau extraction rides the DVE slack inside the loop
            um = gru.tile([16, 16], dt.float32, tag="um", name=f"um{tau}")
            nc.vector.tensor_tensor(out=um[:], in0=Wt[tau + 1][:],
                                    in1=gi("LINW"), op=mybir.AluOpType.mult)
            nc.vector.tensor_reduce(out=u_cols[:, tau : tau + 1], in_=um[:],
                                    axis=mybir.AxisListType.X,
                                    op=mybir.AluOpType.add)

        usm = small.tile([16, T], dt.float32)
        nc.vector.tensor_tensor(out=usm[:], in0=u_cols[:], in1=gi("SEL8"),
                                op=mybir.AluOpType.mult)
        u_sel = small.tile([16, 1], dt.float32)
        nc.vector.tensor_reduce(out=u_sel[:], in_=usm[:],
                                axis=mybir.AxisListType.X,
                                op=mybir.AluOpType.add)
        diag_u = small.tile([16, 16], dt.float32)
        nc.vector.tensor_scalar_mul(diag_u[:], gi("I16"), u_sel[:])
        ones16x128 = small.tile([16, P], dt.float32)
        nc.vector.memset(ones16x128[:], 1.0)
        ub_ps = ps1.tile([P, 16], dt.float32, tag="ub", name="ub_ps")
        nc.tensor.matmul(ub_ps[:], ones16x128[:], diag_u[:], start=True, stop=True)
        ub = small.tile([P, 16], dt.float32)
        nc.scalar.copy(ub[:], ub_ps[:])

        # s = X @ u: 16 ts_mul partials (4x bf16 mode) + pairwise add tree
        pt = [sp.tile([P, RANKS], bf, tag=f"pt{f}", name=f"pt{f}")
              for f in range(F0)]
        for f in range(F0):
            if f >= 6:  # ScalarE computes 10 partials in parallel with DVE
                nc.scalar.activation(pt[f][:], xtb[f // 4][:, f % 4, :],
                                     mybir.ActivationFunctionType.Copy,
                                     scale=ub[:, f : f + 1])
            else:
                nc.vector.tensor_scalar_mul(pt[f][:], xtb[f // 4][:, f % 4, :],
                                            ub[:, f : f + 1])
        lvl = pt
        li = 0
        while len(lvl) > 1:
            nxt = []
            for i in range(0, len(lvl), 2):
                o = sp.tile([P, RANKS], bf, tag=f"tr{li}_{i}", name=f"tr{li}_{i}")
                nc.vector.tensor_tensor(out=o[:], in0=lvl[i][:],
                                        in1=lvl[i + 1][:],
                                        op=mybir.AluOpType.add)
                nxt.append(o)
            lvl = nxt
            li += 1
        s_t = lvl[0]
        nc.sync.dma_start(s_ap[:], s_t[:])
    nc.compile()
    return nc


# ---------------------------------------------------------------- launch 3
def _build_p3(Ls, chunks, f_pad):
    """bf16 combined sg||val stream; per chunk: one DMA, DVE mult (2x),
    per-run fold (L -> L/2, 2x) then reduce (1x). Ls are %4 so folds stay
    4B-aligned and bf16 ops keep their packed perf mode."""
    nc = bacc.Bacc("TRN2", target_bir_lowering=False, debug=False)
    bf = dt.bfloat16
    tot2 = sum(sum(L * cnt for (L, cnt, _) in runs) for _, runs in chunks) * P * 2
    sv_ap = nc.dram_tensor("sgval", [tot2], bf, kind="ExternalInput").ap()
    b_ap = nc.dram_tensor("linb", [P, 1], dt.float32, kind="ExternalInput").ap()
    y_ap = nc.dram_tensor("y", [P, RANKS], dt.float32, kind="ExternalOutput").ap()

    with tile.TileContext(nc) as tc, ExitStack() as ctx:
        io = ctx.enter_context(tc.tile_pool(name="io", bufs=3))
        yp = ctx.enter_context(tc.tile_pool(name="y", bufs=1))
        b_t = yp.tile([P, 1], dt.float32)
        nc.scalar.dma_start(b_t[:], b_ap[:])
        y_t = yp.tile([P, RANKS], dt.float32)
        off = 0
        for ci, (col0, runs) in enumerate(chunks):
            C = sum(L * cnt for (L, cnt, _) in runs)
            comb = io.tile([P, 2 * C], bf, tag="comb", name="comb_t")
            nc.sync.dma_start(
                comb[:], sv_ap[off : off + P * 2 * C].rearrange(
                    "(p j) -> p j", j=2 * C))
            off += P * 2 * C
            w_t = io.tile([P, C], bf, tag="w", name="w_t")
            nc.vector.tensor_tensor(out=w_t[:], in0=comb[:, 0:C],
                                    in1=comb[:, C : 2 * C],
                                    op=mybir.AluOpType.mult)
            f_t = io.tile([P, C // 2], bf, tag="f", name="f_t")
            c = 0
            for L, cnt, rank0 in runs:
                L2 = L // 2
                w3 = w_t[:, c : c + cnt * L].rearrange("p (r l) -> p r l", l=L)
                fo = f_t[:, c // 2 : c // 2 + cnt * L2].rearrange(
                    "p (r l) -> p r l", l=L2)
                nc.vector.tensor_tensor(out=fo, in0=w3[:, :, 0:L2],
                                        in1=w3[:, :, L2:L],
                                        op=mybir.AluOpType.add)
                nc.vector.tensor_reduce(
                    out=y_t[:, rank0 : rank0 + cnt], in_=fo,
                    axis=mybir.AxisListType.X, op=mybir.AluOpType.add,
                )
                c += cnt * L
        yb = yp.tile([P, RANKS], dt.float32)
        nc.vector.tensor_scalar_add(yb[:], y_t[:], b_t[:])
        nc.sync.dma_start(y_ap[:], yb[:])
    nc.compile()
    return nc


# ------------------------------------------------------------ host layout
def _edge_layout(edge_row, edge_col, edge_val):
    """Degree-sorted, rank-equalized destination layout shared across T."""
    degs = np.zeros((T, N_PAD), np.int64)
    orders = np.zeros((T, N_PAD), np.int64)
    for t in range(T):
        deg = np.bincount(edge_row[t].astype(np.int64), minlength=N_PAD)
        degs[t] = deg
        orders[t] = np.argsort(-deg, kind="stable")
    rank_max = np.zeros((T, RANKS), np.int64)
    for t in range(T):
        rank_max[t] = degs[t][orders[t]].reshape(RANKS, P).max(1)
    Ls = rank_max.max(0)
    Ls = np.maximum.accumulate(Ls[::-1])[::-1]  # enforce non-increasing
    Ls = ((np.maximum(Ls, 1) + 3) // 4) * 4  # %4 so the bf16 fold is aligned
    offs = np.zeros(RANKS + 1, np.int64)
    offs[1:] = np.cumsum(Ls)
    f_pad = int(offs[-1])

    col_layout = np.zeros((T, P, f_pad), np.int32)
    val_layout = np.zeros((T, P, f_pad), np.float32)
    for t in range(T):
        row = edge_row[t].astype(np.int64)
        order = orders[t]
        slot_of_node = np.empty(N_PAD, np.int64)
        slot_of_node[order] = np.arange(N_PAD)
        ord_e = np.argsort(row, kind="stable")
        rows_s = row[ord_e]
        deg = degs[t]
        node_start = np.zeros(N_PAD, np.int64)
        node_start[1:] = np.cumsum(deg)[:-1]
        k = np.arange(E, dtype=np.int64) - node_start[rows_s]
        s = slot_of_node[rows_s]
        p_idx = s % P
        r_idx = s // P
        pos = offs[r_idx] + k
        col_layout[t, p_idx, pos] = edge_col[t][ord_e]
        val_layout[t, p_idx, pos] = edge_val[t][ord_e]

    # chunk schedule shared across cores
    FC = 4352
    chunks = []
    cur, cur_cols, col0, r = [], 0, 0, 0
    while r < RANKS:
        L = int(Ls[r])
        cnt = 0
        while r + cnt < RANKS and Ls[r + cnt] == L and cur_cols + (cnt + 1) * L <= FC:
            cnt += 1
        if cnt == 0:
            chunks.append((col0, cur))
            col0 += cur_cols
            cur, cur_cols = [], 0
            continue
        cur.append((L, cnt, r))
        cur_cols += cnt * L
        r += cnt
    if cur:
        chunks.append((col0, cur))
    # split the final chunk so the post-DMA compute tail is short
    if len(chunks) > 1 and sum(L * n for (L, n, _) in chunks[-1][1]) > 1600:
        col0, runs = chunks.pop()
        half = sum(L * n for (L, n, _) in runs) // 2
        a, b, acc = [], [], 0
        for L, n, r0 in runs:
            if acc >= half:
                b.append((L, n, r0))
                continue
            take = min(n, max(1, (half - acc) // L))
            a.append((L, take, r0))
            acc += take * L
            if take < n:
                b.append((L, n - take, r0 + take))
        chunks.append((col0, a))
        chunks.append((col0 + acc, b))
    return Ls, offs, f_pad, col_layout, val_layout, orders, chunks


# ------------------------------------------------------------------ kernel
def kernel(**inputs):
    inp = {k: np.asarray(v) for k, v in inputs.items()}
    X = inp["X"].astype(np.float32, copy=False)  # [T, N, F0]
    edge_row = inp["edge_row"]
    edge_col = inp["edge_col"]
    edge_val = inp["edge_val"].astype(np.float32, copy=False)
    p = inp["p"].astype(np.float32, copy=False)

    # padded, partition-major, feature-transposed X per core:
    # node n = p*RANKS + i;  XT[core t][p, f*RANKS + i] = X[t, n, f]
    X_pad = np.zeros((T, N_PAD, F0), np.float32)
    X_pad[:, :N] = X
    XT_core = np.ascontiguousarray(
        X_pad.reshape(T, P, RANKS, F0).transpose(0, 1, 3, 2)
    ).reshape(T, P, F0 * RANKS)

    Ls, offs, f_pad, col_layout, val_layout, orders, chunks = _edge_layout(
        edge_row, edge_col, edge_val
    )

    # ---- launch 1: yraw_t = X_t @ p
    if "p1" not in _cache:
        _cache["p1"] = _build_p1()
    p_rep = np.tile(p[None, :], (P, 1))
    in1 = [{"XT": XT_core[t], "prep": p_rep} for t in range(T)]
    res1 = _run(_cache["p1"], in1)
    yraw = np.stack([res1.results[t]["yraw"].reshape(-1) for t in range(T)])

    # ---- host: top-16 indices (index selection only)
    yraw16 = np.zeros((16, T), np.float32)
    X16 = np.zeros((16, T * F0), np.float32)
    for t in range(T):
        y = yraw[t][:N]
        cand = np.argpartition(y, -32)[-32:]
        order = cand[np.lexsort((cand, -y[cand]))][:16]
        yraw16[:, t] = y[order]
        X16[:, t * F0 : (t + 1) * F0] = X[t][order]

    # ---- launch 2: GRU chain + s_t = X_t @ (W_t @ lin_w)
    if "p2" not in _cache:
        _cache["p2"] = _build_p2()
    f32 = np.float32
    BF = mybir.dt.np(dt.bfloat16)
    smalls = np.zeros((16, SMALLS_W), f32)

    def put(name, arr):
        a, b = _COLS[name]
        arr = np.asarray(arr, f32)
        smalls[0 : arr.shape[0], a:b] = arr

    put("UZT", inp["U_Z"].T)
    put("WZT", inp["W_Z"].T)
    put("BZR8", np.tile(np.concatenate([inp["B_Z"], inp["B_R"]], axis=1), (1, T)))
    put("URT", inp["U_R"].T)
    put("WRT", inp["W_R"].T)
    put("UHT", inp["U_H"].T)
    put("WHT", inp["W_H"].T)
    put("BH8", np.tile(np.asarray(inp["B_H"], f32), (1, T)))
    put("X16", X16)
    put("YR16", yraw16)
    put("I16", np.eye(16, dtype=f32))
    put("WINIT", inp["W_init"].astype(f32))
    put("LINW", np.tile(inp["lin_w"].astype(f32)[None, :], (16, 1)))
    put("PREP", np.tile(p[None, :], (16, 1)))
    XTB = np.ascontiguousarray(XT_core.astype(BF))
    in2 = []
    for t in range(T):
        sm_t = smalls.copy()
        a, b = _COLS["SEL8"]
        sm_t[0:16, a + t] = 1.0
        in2.append({"XTB": XTB[t], "smalls": sm_t})
    res2 = _run(_cache["p2"], in2)
    s_all = np.stack(
        [np.asarray(res2.results[t]["s"]).reshape(-1) for t in range(T)])  # bf16

    # ---- host re-staging: gather s into the edge layout (index move only);
    # per chunk the sg and val planes interleave into one [P, 2C] block so
    # L3 does a single DMA per chunk
    val_bf = val_layout.astype(BF)
    sgval = []
    for t in range(T):
        sg_t = s_all[t][col_layout[t]]  # [P, f_pad] bf16 gather
        parts = []
        for c0, runs in chunks:
            C = sum(L * n for (L, n, _) in runs)
            comb = np.empty((P, 2 * C), BF)
            comb[:, :C] = sg_t[:, c0 : c0 + C]
            comb[:, C:] = val_bf[t][:, c0 : c0 + C]
            parts.append(comb.reshape(-1))
        sgval.append(np.concatenate(parts))

    # ---- launch 3: w = val*sg, fold+segmented reduce per rank, + lin_b
    key3 = ("p3", f_pad, tuple(Ls.tolist()))
    if key3 not in _cache:
        _cache[key3] = _build_p3(Ls, chunks, f_pad)
    b_rep = np.full((P, 1), np.float32(inp["lin_b"][0]), np.float32)
    in3 = [{"sgval": sgval[t], "linb": b_rep} for t in range(T)]
    res3 = _run(_cache[key3], in3)

    # ---- host: un-permute ranks back to node ids
    out = np.zeros((T, N), np.float32)
    for t in range(T):
        y3 = res3.results[t]["y"]  # [P, RANKS]; slot s=128r+p -> y3[p, r]
        flat = np.ascontiguousarray(y3.T).reshape(-1)
        full = np.empty(N_PAD, np.float32)
        full[orders[t]] = flat
        out[t] = full[:N]
    return out



# revision 27
# speedup vs baseline: 1.0171x; 1.0171x over previous
"""EvolveGCN-reg Trainium2 kernel (8 NeuronCores, timestep-parallel).

Math: the reference computes, per timestep t (scan carrying a GRU-evolved
16x16 weight W):
    y_t   = X_t @ p / ||p||;  (yk, idx) = top16(y_t);  Xs = (X_t[idx] * yk).T
    W_t   = GRU(W_{t-1}, Xs)          (16x16 matmuls, tiny)
    AH    = segment_sum(val * X_t[col], row, N)       (3.2M-edge sparse op)
    out_t = (AH @ W_t) @ lin_w + b
Key identity used here:  out_t = A_t @ (X_t @ (W_t @ lin_w)) + b, so the
feature dimension collapses and the sparse phase is a scalar gather /
segment-sum:  out_t[n] = b + sum_{e: row[e]=n} val[e] * s_t[col[e]],
with s_t = X_t @ u_t and u_t = W_t @ lin_w.

Sharding: core t owns timestep t (uniform load, no collectives). Host does
index-space layout only (the sharding hint's "partition edge lists by
destination range" taken to its conclusion): edges are grouped by
destination, destinations degree-sorted and assigned round-robin to the 128
SBUF partitions so each "rank" of 128 nodes shares a common padded segment
length (padded %4 so bf16 folds stay 4B-aligned). The one index permutation
(s gathered edge-wise) happens during host-side re-staging between device
launches; every floating-point operation of the model runs on the
NeuronCores.

Device launches (all math on device):
  L1: yraw_t = X_t @ p in fp32             -> host extracts top-16 *indices*
      (selection needs fp32-exact y: adjacent top-16 gaps are ~5e-5)
  L2: ||p|| normalize, Xs_tau, GRU chain, u_t select, s_t = X_t @ u_t.
      X streamed as bf16 (halves traffic); gate accumulators are two wide
      PSUM tiles preloaded with biases + W_gate@Xs early, so each tau only
      runs U@W -> sigmoid -> RW -> U_H@RW -> tanh -> 2 DVE ops; the s
      matvec splits 10 ACT-copy-scale partials against 6 DVE ts_mul + add
      tree; s returned as bf16.
  L3: one bf16 sg||val DMA per chunk, DVE mult (2x packed) + fold (L->L/2)
      + segmented reduce, + b. DMA runs at HBM line rate (~340 GB/s).
"""

import numpy as np
from contextlib import ExitStack

import concourse.bass as bass
import concourse.bacc as bacc
import concourse.tile as tile
from concourse import mybir
from concourse.bass_utils import run_bass_kernel_spmd

dt = mybir.dt

T, N, E, F0, F1 = 8, 100000, 3200000, 16, 16
NCORES = 8
P = 128
RANKS = (N + P - 1) // P  # 782
N_PAD = P * RANKS  # 100096
CORE_IDS = list(range(NCORES))

_cache = {}


def _axon_reset():
    try:
        import ctypes

        lib = ctypes.CDLL("/opt/axon/libaxon_pjrt.so")
        lib.axon_reset.restype = ctypes.c_int64
        lib.axon_reset()
    except Exception:
        pass


def _run(nc, in_maps):
    try:
        return run_bass_kernel_spmd(nc, in_maps, core_ids=CORE_IDS)
    except Exception:
        _axon_reset()
        return run_bass_kernel_spmd(nc, in_maps, core_ids=CORE_IDS)

GP_FEATS = (4, 5, 6, 7)  # features accumulated on GpSimd (rest on DVE)


def _emit_matvec(nc, io, acc_pool, xt_ap, u_sb, out_sb):
    """out_sb[p, r] = sum_f X_T[p, f, r] * u_sb[p, f]; X_T streamed in 8
    feature-chunks, fused multiply-accumulate chain on DVE."""
    xt = io.tile([P, F0, RANKS], dt.float32, tag="xt", name="xt")
    FC = F0 // 8
    for c4 in range(8):
        nc.sync.dma_start(
            xt[:, c4 * FC : (c4 + 1) * FC, :],
            xt_ap[:, c4 * FC * RANKS : (c4 + 1) * FC * RANKS].rearrange(
                "p (f r) -> p f r", r=RANKS),
        )
    # two interleaved accumulator chains hide DVE issue gaps
    acc_a = acc_pool.tile([P, RANKS], dt.float32, tag="acc_a", name="acc_a")
    acc_b = acc_pool.tile([P, RANKS], dt.float32, tag="acc_b", name="acc_b")
    nc.vector.tensor_scalar_mul(acc_a[:], xt[:, 0, :], u_sb[:, 0:1])
    nc.vector.tensor_scalar_mul(acc_b[:], xt[:, 1, :], u_sb[:, 1:2])
    for f in range(2, F0):
        acc = acc_a if f % 2 == 0 else acc_b
        nc.vector.scalar_tensor_tensor(
            out=acc[:], in0=xt[:, f, :], scalar=u_sb[:, f : f + 1],
            in1=acc[:], op0=mybir.AluOpType.mult, op1=mybir.AluOpType.add,
        )
    nc.vector.tensor_tensor(out=out_sb[:], in0=acc_a[:], in1=acc_b[:],
                            op=mybir.AluOpType.add)


# ---------------------------------------------------------------- launch 1
def _build_p1():
    nc = bacc.Bacc("TRN2", target_bir_lowering=False, debug=False)
    xt_ap = nc.dram_tensor("XT", [P, F0 * RANKS], dt.float32, kind="ExternalInput").ap()
    prep_ap = nc.dram_tensor("prep", [P, F0], dt.float32, kind="ExternalInput").ap()
    y_ap = nc.dram_tensor("yraw", [P, RANKS], dt.float32, kind="ExternalOutput").ap()

    with tile.TileContext(nc) as tc, ExitStack() as ctx:
        io = ctx.enter_context(tc.tile_pool(name="io", bufs=1))
        yp = ctx.enter_context(tc.tile_pool(name="y", bufs=1))
        p_t = yp.tile([P, F0], dt.float32)
        nc.scalar.dma_start(p_t[:], prep_ap[:])
        y_t = yp.tile([P, RANKS], dt.float32)
        _emit_matvec(nc, io, yp, xt_ap, p_t, y_t)
        nc.sync.dma_start(y_ap[:], y_t[:])
    nc.compile()
    return nc


# ---------------------------------------------------------------- launch 2
# packed small-input column layout (16 partition rows). Gate accumulators
# are PSUM-preloaded with the raw bias B (ACT copy), then two K=16 matmuls
# accumulate U@W and W@Xs on top -- no I16 bias matmul, no stacked rhs.
_COLS = {}
_off = 0
for _n, _w in [("UZT", 16), ("WZT", 16), ("BZR8", 256), ("URT", 16), ("WRT", 16),
               ("UHT", 16), ("WHT", 16), ("BH8", 128), ("X16", 128),
               ("YR16", 8), ("I16", 16), ("WINIT", 16), ("LINW", 16),
               ("SEL8", 8), ("PREP", 16)]:
    _COLS[_n] = (_off, _off + _w)
    _off += _w
SMALLS_W = _off


def _build_p2():
    nc = bacc.Bacc("TRN2", target_bir_lowering=False, debug=False)
    bf = dt.bfloat16
    xt_ap = nc.dram_tensor("XTB", [P, F0 * RANKS], bf, kind="ExternalInput").ap()
    sm_ap = nc.dram_tensor("smalls", [16, SMALLS_W], dt.float32, kind="ExternalInput").ap()
    s_ap = nc.dram_tensor("s", [P, RANKS], bf, kind="ExternalOutput").ap()

    with tile.TileContext(nc) as tc, ExitStack() as ctx:
        small = ctx.enter_context(tc.tile_pool(name="small", bufs=1))
        gru = ctx.enter_context(tc.tile_pool(name="gru", bufs=2))
        rhp = ctx.enter_context(tc.tile_pool(name="rhp", bufs=1))
        ps = ctx.enter_context(tc.tile_pool(name="ps", bufs=3, space="PSUM"))
        ps1 = ctx.enter_context(tc.tile_pool(name="ps1", bufs=1, space="PSUM"))
        io = ctx.enter_context(tc.tile_pool(name="io", bufs=1))
        sp = ctx.enter_context(tc.tile_pool(name="s", bufs=1))

        sm = small.tile([16, SMALLS_W], dt.float32)
        nc.scalar.dma_start(sm[:], sm_ap[:])

        # X (bf16) streamed in 4 feature chunks; queue the DMAs up front
        FCH = 4
        xtb = []
        for c in range(F0 // FCH):
            xt = io.tile([P, FCH, RANKS], bf, tag=f"xt{c}", name=f"xt{c}")
            nc.sync.dma_start(
                xt[:],
                xt_ap[:, c * FCH * RANKS : (c + 1) * FCH * RANKS].rearrange(
                    "p (f r) -> p f r", r=RANKS),
            )
            xtb.append(xt)

        def gi(name, rows=16):
            a, b = _COLS[name]
            return sm[0:rows, a:b]

        # invp = 1/||p||. square on DVE; sqrt FIRST so its ACT table load
        # lands before the sigmoid/tanh warms (table thrash costs 1.3us)
        psq = small.tile([1, F0], dt.float32)
        nc.vector.tensor_tensor(out=psq[:], in0=gi("PREP")[0:1, :],
                                in1=gi("PREP")[0:1, :], op=mybir.AluOpType.mult)
        pss = small.tile([1, 1], dt.float32)
        nc.vector.tensor_reduce(out=pss[:], in_=psq[:], axis=mybir.AxisListType.X,
                                op=mybir.AluOpType.add)
        pnorm = small.tile([1, 1], dt.float32)
        nc.scalar.sqrt(pnorm[:], pss[:])
        invp = small.tile([1, 1], dt.float32)
        nc.vector.reciprocal(invp[:], pnorm[:])
        # prefetch the chain's ACT tables; reading pnorm forces these AFTER
        # sqrt so its table load cannot evict sigmoid/tanh mid-kernel
        warm = small.tile([1, 2], dt.float32)
        nc.scalar.activation(warm[:, 0:1], pnorm[:],
                             mybir.ActivationFunctionType.Sigmoid)
        nc.scalar.activation(warm[:, 1:2], pnorm[:],
                             mybir.ActivationFunctionType.Tanh)
        ones1x16 = small.tile([1, 16], dt.float32)
        nc.vector.memset(ones1x16[:], 1.0)
        invp16_ps = ps1.tile([16, 1], dt.float32, tag="misc", name="invp16_ps")
        nc.tensor.matmul(invp16_ps[:], ones1x16[:], invp[:], start=True, stop=True)
        invp16 = small.tile([16, 1], dt.float32)
        nc.scalar.copy(invp16[:], invp16_ps[:])

        # W chain tiles + Xs tiles, all plain base-0 [16,16]
        Wt = [rhp.tile([16, 16], dt.float32, tag=f"w{t}", name=f"w{t}")
              for t in range(T + 1)]
        Xs = [rhp.tile([16, 16], dt.float32, tag=f"xs{t}", name=f"xs{t}")
              for t in range(T)]
        nc.scalar.copy(Wt[0][:], gi("WINIT"))

        # Xs_tau = X16_tau^T @ (I * yraw_tau * invp)
        for tau in range(T):
            dg = gru.tile([16, 16], dt.float32, tag="diag", name=f"dg{tau}")
            nc.vector.tensor_scalar(
                out=dg[:], in0=gi("I16"), scalar1=gi("YR16")[:, tau : tau + 1],
                op0=mybir.AluOpType.mult, scalar2=invp16[:],
                op1=mybir.AluOpType.mult)
            xs_ps = ps.tile([16, 16], dt.float32, tag="m16", name=f"xsps{tau}")
            nc.tensor.matmul(xs_ps[:],
                             gi("X16")[:, tau * F0 : (tau + 1) * F0],
                             dg[:], start=True, stop=True)
            nc.scalar.copy(Xs[tau][:], xs_ps[:])

        # GRU chain. All-tau PSUM accumulators: azr [16, 8*32] (Z|R per
        # tau), ah [16, 8*16]. Bias + W_gate@Xs parts are accumulated EARLY
        # (during the X DMA); per tau only U@W / U@RW + activation remain.
        SIG = mybir.ActivationFunctionType.Sigmoid
        TANH = mybir.ActivationFunctionType.Tanh
        u_cols = small.tile([16, T], dt.float32)
        azr = ps1.tile([16, T * 32], dt.float32, tag="azr", name="azr")
        ah = ps1.tile([16, T * 16], dt.float32, tag="ah", name="ah")
        nc.vector.tensor_copy(azr[:], gi("BZR8"))
        nc.vector.tensor_copy(ah[:], gi("BH8"))
        for tau in range(T):
            z0 = tau * 32
            nc.tensor.matmul(azr[:, z0 : z0 + 16], gi("WZT"), Xs[tau][:],
                             start=False, stop=False, skip_group_check=True)
            nc.tensor.matmul(azr[:, z0 + 16 : z0 + 32], gi("WRT"), Xs[tau][:],
                             start=False, stop=False, skip_group_check=True)
            nc.tensor.matmul(ah[:, tau * 16 : (tau + 1) * 16], gi("WHT"),
                             Xs[tau][:], start=False, stop=False,
                             skip_group_check=True)
        for tau in range(T):
            z0 = tau * 32
            nc.tensor.matmul(azr[:, z0 : z0 + 16], gi("UZT"), Wt[tau][:],
                             start=False, stop=False, skip_group_check=True)
            nc.tensor.matmul(azr[:, z0 + 16 : z0 + 32], gi("URT"), Wt[tau][:],
                             start=False, stop=True, skip_group_check=True)
            ZR = gru.tile([16, 32], dt.float32, tag="gzr", name=f"gzr{tau}")
            nc.scalar.activation(ZR[:], azr[:, z0 : z0 + 32], SIG)
            RW = gru.tile([16, 16], dt.float32, tag="rw", name=f"rw{tau}")
            nc.vector.tensor_tensor(out=RW[:], in0=ZR[:, 16:32], in1=Wt[tau][:],
                                    op=mybir.AluOpType.mult)
            nc.tensor.matmul(ah[:, tau * 16 : (tau + 1) * 16], gi("UHT"),
                             RW[:], start=False, stop=True,
                             skip_group_check=True)
            # W_new = (W - Zg*W) + Zg*Ht; the first half is ready right
            # after sigmoid, so only two DVE ops trail the tanh
            zw = gru.tile([16, 16], dt.float32, tag="zw", name=f"zw{tau}")
            nc.vector.tensor_tensor(out=zw[:], in0=ZR[:, 0:16], in1=Wt[tau][:],
                                    op=mybir.AluOpType.mult)
            wm = gru.tile([16, 16], dt.float32, tag="wm", name=f"wm{tau}")
            nc.vector.tensor_tensor(out=wm[:], in0=Wt[tau][:], in1=zw[:],
                                    op=mybir.AluOpType.subtract)
            Ht = gru.tile([16, 16], dt.float32, tag="ht", name=f"ht{tau}")
            nc.scalar.activation(Ht[:], ah[:, tau * 16 : (tau + 1) * 16], TANH)
            zh = gru.tile([16, 16], dt.float32, tag="zh", name=f"zh{tau}")
            nc.vector.tensor_tensor(out=zh[:], in0=ZR[:, 0:16], in1=Ht[:],
                                    op=mybir.AluOpType.mult)
            nc.vector.tensor_tensor(out=Wt[tau + 1][:], in0=wm[:],
                                    in1=zh[:], op=mybir.AluOpType.add)
            # u_tau extraction rides the DVE slack inside the loop
            um = gru.tile([16, 16], dt.float32, tag="um", name=f"um{tau}")
            nc.vector.tensor_tensor(out=um[:], in0=Wt[tau + 1][:],
                                    in1=gi("LINW"), op=mybir.AluOpType.mult)
            nc.vector.tensor_reduce(out=u_cols[:, tau : tau + 1], in_=um[:],
                                    axis=mybir.AxisListType.X,
                                    op=mybir.AluOpType.add)

        usm = small.tile([16, T], dt.float32)
        nc.vector.tensor_tensor(out=usm[:], in0=u_cols[:], in1=gi("SEL8"),
                                op=mybir.AluOpType.mult)
        u_sel = small.tile([16, 1], dt.float32)
        nc.vector.tensor_reduce(out=u_sel[:], in_=usm[:],
                                axis=mybir.AxisListType.X,
                                op=mybir.AluOpType.add)
        diag_u = small.tile([16, 16], dt.float32)
        nc.vector.tensor_scalar_mul(diag_u[:], gi("I16"), u_sel[:])
        ones16x128 = small.tile([16, P], dt.float32)
        nc.vector.memset(ones16x128[:], 1.0)
        ub_ps = ps1.tile([P, 16], dt.float32, tag="ub", name="ub_ps")
        nc.tensor.matmul(ub_ps[:], ones16x128[:], diag_u[:], start=True, stop=True)
        ub = small.tile([P, 16], dt.float32)
        nc.scalar.copy(ub[:], ub_ps[:])

        # s = X @ u: 16 ts_mul partials (4x bf16 mode) + pairwise add tree
        pt = [sp.tile([P, RANKS], bf, tag=f"pt{f}", name=f"pt{f}")
              for f in range(F0)]
        for f in range(F0):
            if f >= 6:  # ScalarE computes 10 partials in parallel with DVE
                nc.scalar.activation(pt[f][:], xtb[f // 4][:, f % 4, :],
                                     mybir.ActivationFunctionType.Copy,
                                     scale=ub[:, f : f + 1])
            else:
                nc.vector.tensor_scalar_mul(pt[f][:], xtb[f // 4][:, f % 4, :],
                                            ub[:, f : f + 1])
        lvl = pt
        li = 0
        while len(lvl) > 1:
            nxt = []
            for i in range(0, len(lvl), 2):
                o = sp.tile([P, RANKS], bf, tag=f"tr{li}_{i}", name=f"tr{li}_{i}")
                nc.vector.tensor_tensor(out=o[:], in0=lvl[i][:],
                                        in1=lvl[i + 1][:],
                                        op=mybir.AluOpType.add)
                nxt.append(o)
            lvl = nxt
            li += 1
        s_t = lvl[0]
        nc.sync.dma_start(s_ap[:], s_t[:])
    nc.compile()
    return nc


# ---------------------------------------------------------------- launch 3
def _build_p3(Ls, chunks, f_pad):
    """bf16 combined sg||val stream; per chunk: one DMA, DVE mult (2x),
    per-run fold (L -> L/2, 2x) then reduce (1x). Ls are %4 so folds stay
    4B-aligned and bf16 ops keep their packed perf mode."""
    nc = bacc.Bacc("TRN2", target_bir_lowering=False, debug=False)
    bf = dt.bfloat16
    tot2 = sum(sum(L * cnt for (L, cnt, _) in runs) for _, runs in chunks) * P * 2
    sv_ap = nc.dram_tensor("sgval", [tot2], bf, kind="ExternalInput").ap()
    b_ap = nc.dram_tensor("linb", [P, 1], dt.float32, kind="ExternalInput").ap()
    y_ap = nc.dram_tensor("y", [P, RANKS], dt.float32, kind="ExternalOutput").ap()

    with tile.TileContext(nc) as tc, ExitStack() as ctx:
        io = ctx.enter_context(tc.tile_pool(name="io", bufs=3))
        yp = ctx.enter_context(tc.tile_pool(name="y", bufs=1))
        b_t = yp.tile([P, 1], dt.float32)
        nc.scalar.dma_start(b_t[:], b_ap[:])
        y_t = yp.tile([P, RANKS], dt.float32)
        off = 0
        for ci, (col0, runs) in enumerate(chunks):
            C = sum(L * cnt for (L, cnt, _) in runs)
            comb = io.tile([P, 2 * C], bf, tag="comb", name="comb_t")
            nc.sync.dma_start(
                comb[:], sv_ap[off : off + P * 2 * C].rearrange(
                    "(p j) -> p j", j=2 * C))
            off += P * 2 * C
            w_t = io.tile([P, C], bf, tag="w", name="w_t")
            nc.vector.tensor_tensor(out=w_t[:], in0=comb[:, 0:C],
                                    in1=comb[:, C : 2 * C],
                                    op=mybir.AluOpType.mult)
            f_t = io.tile([P, C // 2], bf, tag="f", name="f_t")
            c = 0
            for L, cnt, rank0 in runs:
                L2 = L // 2
                w3 = w_t[:, c : c + cnt * L].rearrange("p (r l) -> p r l", l=L)
                fo = f_t[:, c // 2 : c // 2 + cnt * L2].rearrange(
                    "p (r l) -> p r l", l=L2)
                nc.vector.tensor_tensor(out=fo, in0=w3[:, :, 0:L2],
                                        in1=w3[:, :, L2:L],
                                        op=mybir.AluOpType.add)
                nc.vector.tensor_reduce(
                    out=y_t[:, rank0 : rank0 + cnt], in_=fo,
                    axis=mybir.AxisListType.X, op=mybir.AluOpType.add,
                )
                c += cnt * L
        yb = yp.tile([P, RANKS], dt.float32)
        nc.vector.tensor_scalar_add(yb[:], y_t[:], b_t[:])
        nc.sync.dma_start(y_ap[:], yb[:])
    nc.compile()
    return nc


# ------------------------------------------------------------ host layout
def _edge_layout(edge_row, edge_col, edge_val):
    """Degree-sorted, rank-equalized destination layout shared across T."""
    degs = np.zeros((T, N_PAD), np.int64)
    orders = np.zeros((T, N_PAD), np.int64)
    for t in range(T):
        deg = np.bincount(edge_row[t].astype(np.int64), minlength=N_PAD)
        degs[t] = deg
        orders[t] = np.argsort(-deg, kind="stable")
    rank_max = np.zeros((T, RANKS), np.int64)
    for t in range(T):
        rank_max[t] = degs[t][orders[t]].reshape(RANKS, P).max(1)
    Ls = rank_max.max(0)
    Ls = np.maximum.accumulate(Ls[::-1])[::-1]  # enforce non-increasing
    Ls = ((np.maximum(Ls, 1) + 3) // 4) * 4  # %4 so the bf16 fold is aligned
    offs = np.zeros(RANKS + 1, np.int64)
    offs[1:] = np.cumsum(Ls)
    f_pad = int(offs[-1])

    col_layout = np.zeros((T, P, f_pad), np.int32)
    val_layout = np.zeros((T, P, f_pad), np.float32)
    for t in range(T):
        row = edge_row[t].astype(np.int64)
        order = orders[t]
        slot_of_node = np.empty(N_PAD, np.int64)
        slot_of_node[order] = np.arange(N_PAD)
        ord_e = np.argsort(row, kind="stable")
        rows_s = row[ord_e]
        deg = degs[t]
        node_start = np.zeros(N_PAD, np.int64)
        node_start[1:] = np.cumsum(deg)[:-1]
        k = np.arange(E, dtype=np.int64) - node_start[rows_s]
        s = slot_of_node[rows_s]
        p_idx = s % P
        r_idx = s // P
        pos = offs[r_idx] + k
        col_layout[t, p_idx, pos] = edge_col[t][ord_e]
        val_layout[t, p_idx, pos] = edge_val[t][ord_e]

    # chunk schedule shared across cores
    FC = 4352
    chunks = []
    cur, cur_cols, col0, r = [], 0, 0, 0
    while r < RANKS:
        L = int(Ls[r])
        cnt = 0
        while r + cnt < RANKS and Ls[r + cnt] == L and cur_cols + (cnt + 1) * L <= FC:
            cnt += 1
        if cnt == 0:
            chunks.append((col0, cur))
            col0 += cur_cols
            cur, cur_cols = [], 0
            continue
        cur.append((L, cnt, r))
        cur_cols += cnt * L
        r += cnt
    if cur:
        chunks.append((col0, cur))
    # split the final chunk so the post-DMA compute tail is short
    if len(chunks) > 1 and sum(L * n for (L, n, _) in chunks[-1][1]) > 1600:
        col0, runs = chunks.pop()
        half = sum(L * n for (L, n, _) in runs) // 2
        a, b, acc = [], [], 0
        for L, n, r0 in runs:
            if acc >= half:
                b.append((L, n, r0))
                continue
            take = min(n, max(1, (half - acc) // L))
            a.append((L, take, r0))
            acc += take * L
            if take < n:
                b.append((L, n - take, r0 + take))
        chunks.append((col0, a))
        chunks.append((col0 + acc, b))
    return Ls, offs, f_pad, col_layout, val_layout, orders, chunks


# ------------------------------------------------------------------ kernel
def kernel(**inputs):
    inp = {k: np.asarray(v) for k, v in inputs.items()}
    X = inp["X"].astype(np.float32, copy=False)  # [T, N, F0]
    edge_row = inp["edge_row"]
    edge_col = inp["edge_col"]
    edge_val = inp["edge_val"].astype(np.float32, copy=False)
    p = inp["p"].astype(np.float32, copy=False)

    # padded, partition-major, feature-transposed X per core:
    # node n = p*RANKS + i;  XT[core t][p, f*RANKS + i] = X[t, n, f]
    X_pad = np.zeros((T, N_PAD, F0), np.float32)
    X_pad[:, :N] = X
    XT_core = np.ascontiguousarray(
        X_pad.reshape(T, P, RANKS, F0).transpose(0, 1, 3, 2)
    ).reshape(T, P, F0 * RANKS)

    Ls, offs, f_pad, col_layout, val_layout, orders, chunks = _edge_layout(
        edge_row, edge_col, edge_val
    )

    # ---- launch 1: yraw_t = X_t @ p
    if "p1" not in _cache:
        _cache["p1"] = _build_p1()
    p_rep = np.tile(p[None, :], (P, 1))
    in1 = [{"XT": XT_core[t], "prep": p_rep} for t in range(T)]
    res1 = _run(_cache["p1"], in1)
    yraw = np.stack([res1.results[t]["yraw"].reshape(-1) for t in range(T)])

    # ---- host: top-16 indices (index selection only)
    yraw16 = np.zeros((16, T), np.float32)
    X16 = np.zeros((16, T * F0), np.float32)
    for t in range(T):
        y = yraw[t][:N]
        cand = np.argpartition(y, -32)[-32:]
        order = cand[np.lexsort((cand, -y[cand]))][:16]
        yraw16[:, t] = y[order]
        X16[:, t * F0 : (t + 1) * F0] = X[t][order]

    # ---- launch 2: GRU chain + s_t = X_t @ (W_t @ lin_w)
    if "p2" not in _cache:
        _cache["p2"] = _build_p2()
    f32 = np.float32
    BF = mybir.dt.np(dt.bfloat16)
    smalls = np.zeros((16, SMALLS_W), f32)

    def put(name, arr):
        a, b = _COLS[name]
        arr = np.asarray(arr, f32)
        smalls[0 : arr.shape[0], a:b] = arr

    put("UZT", inp["U_Z"].T)
    put("WZT", inp["W_Z"].T)
    put("BZR8", np.tile(np.concatenate([inp["B_Z"], inp["B_R"]], axis=1), (1, T)))
    put("URT", inp["U_R"].T)
    put("WRT", inp["W_R"].T)
    put("UHT", inp["U_H"].T)
    put("WHT", inp["W_H"].T)
    put("BH8", np.tile(np.asarray(inp["B_H"], f32), (1, T)))
    put("X16", X16)
    put("YR16", yraw16)
    put("I16", np.eye(16, dtype=f32))
    put("WINIT", inp["W_init"].astype(f32))
    put("LINW", np.tile(inp["lin_w"].astype(f32)[None, :], (16, 1)))
    put("PREP", np.tile(p[None, :], (16, 1)))
    XTB = np.ascontiguousarray(XT_core.astype(BF))
    in2 = []
    for t in range(T):
        sm_t = smalls.copy()
        a, b = _COLS["SEL8"]
        sm_t[0:16, a + t] = 1.0
        in2.append({"XTB": XTB[t], "smalls": sm_t})
    res2 = _run(_cache["p2"], in2)
    s_all = np.stack(
        [np.asarray(res2.results[t]["s"]).reshape(-1) for t in range(T)])  # bf16

    # ---- host re-staging: gather s into the edge layout (index move only);
    # per chunk the sg and val planes interleave into one [P, 2C] block so
    # L3 does a single DMA per chunk
    val_bf = val_layout.astype(BF)
    sgval = []
    for t in range(T):
        sg_t = s_all[t][col_layout[t]]  # [P, f_pad] bf16 gather
        parts = []
        for c0, runs in chunks:
            C = sum(L * n for (L, n, _) in runs)
            comb = np.empty((P, 2 * C), BF)
            comb[:, :C] = sg_t[:, c0 : c0 + C]
            comb[:, C:] = val_bf[t][:, c0 : c0 + C]
            parts.append(comb.reshape(-1))
        sgval.append(np.concatenate(parts))

    # ---- launch 3: w = val*sg, fold+segmented reduce per rank, + lin_b
    key3 = ("p3", f_pad, tuple(Ls.tolist()))
    if key3 not in _cache:
        _cache[key3] = _build_p3(Ls, chunks, f_pad)
    b_rep = np.full((P, 1), np.float32(inp["lin_b"][0]), np.float32)
    in3 = [{"sgval": sgval[t], "linb": b_rep} for t in range(T)]
    res3 = _run(_cache[key3], in3)

    # ---- host: un-permute ranks back to node ids
    out = np.zeros((T, N), np.float32)
    for t in range(T):
        y3 = res3.results[t]["y"]  # [P, RANKS]; slot s=128r+p -> y3[p, r]
        flat = np.ascontiguousarray(y3.T).reshape(-1)
        full = np.empty(N_PAD, np.float32)
        full[orders[t]] = flat
        out[t] = full[:N]
    return out

